# revision 4
# baseline (speedup 1.0000x reference)
"""Trainium2 Bass kernel for BodyStructureLoss.

Computes: mean over (B, J) of where(||kps[b,j,:]|| > 1.0, ||kps[b,j,:]||, 0)
for kps of shape [524288, 17, 3] float32.

Strategy (data-parallel over 8 NeuronCores):
  - Each core gets B/8 = 65536 batch rows viewed as [128, 26112] fp32.
  - Tiles of F columns stream in via DMA. Each tile is squared with a
    transposed write into a bf16 tile yt so the three components of each
    (x,y,z) triplet land in separate contiguous thirds:
        yt[:, t*M + m] = xt[:, 3m + t]^2   (M = F/3)
    The squaring engine alternates ACT (activation Square) / DVE
    (tensor_tensor mult) to balance load.
  - Two packed bf16 tensor_tensor adds (DVE 2x mode) produce the
    per-triplet squared norm s [P, M].
  - Per tile: tensor_scalar is_gt (4x bf16) accumulates count(s > 1) into
    accs[:, 2i+1]; ACT sqrt gives d bf16; tensor_scalar max (4x bf16)
    accumulates sum(max(d, 1)) into accs[:, 2i].
  - The tile plan ramps up (612, 1020) and tapers down (1224, 1020, 612,
    408) so the DMA stream stays packed and the post-stream compute drain
    is short.
  - Host sums all partials: sum(accs) = masked_sum + B*J, so subtract
    B*J and divide by B*J.
"""

import os

import numpy as np

# the NTFF trace path needs antenv.axon_hooks, which this client image lacks;
# force-disable so a stray BASS_TRACE=1 in the environment cannot break runs
os.environ["BASS_NEVER_TRACE"] = "1"

import concourse.bacc as bacc
import concourse.mybir as mybir
from concourse.bass_utils import run_bass_kernel_spmd
from concourse.tile import TileContext

B, J, D = 524288, 17, 3
HALF_BODY = 1.0  # threshold/2 with threshold=2.0
N_CORES = 8
P = 128
COLS = (B // N_CORES) * J * D // P  # 26112

_DT = mybir.dt.float32
_BF = mybir.dt.bfloat16


def _default_cfg():
    ramp = [612, 1020]
    tail = [1224, 1020, 612, 408]
    body = COLS - sum(ramp) - sum(tail)
    assert body % 1632 == 0
    tiles = ramp + [1632] * (body // 1632) + tail
    n = len(tiles)
    nt = len(tail)
    sq = {i: ("A" if i % 2 == 0 else "V") for i in range(n - nt)}
    for k, e in enumerate("VAVA"):
        sq[n - nt + k] = e
    return {"tiles": tiles, "sq_eng": sq, "b_lag": 2, "sbufs": 8}


def build_nc(cfg=None):
    if cfg is None:
        cfg = _default_cfg()
    tiles = cfg["tiles"]
    assert sum(tiles) == COLS
    assert all(f % 3 == 0 for f in tiles)
    n_t = len(tiles)
    M_MAX = max(tiles) // 3
    F_PAD = max(tiles)
    sq_eng = cfg.get("sq_eng", {})
    add_eng = cfg.get("add_eng", {})
    count_eng = cfg.get("count_eng", {})
    b_lag = cfg.get("b_lag", 1)
    flush_from = cfg.get("flush_from", None)

    nc = bacc.Bacc(
        "TRN2", target_bir_lowering=False, debug=False, num_devices=N_CORES
    )
    x = nc.dram_tensor("x", [P, COLS], _DT, kind="ExternalInput")
    out = nc.dram_tensor("out", [P, 2 * n_t], _DT, kind="ExternalOutput")

    with TileContext(nc, pool_alloc_mode=cfg.get("pool_mode", "stack")) as tc:
        with (
            tc.tile_pool(name="xin", bufs=cfg.get("xbufs", 6)) as xpool,
            tc.tile_pool(name="ysq", bufs=cfg.get("ybufs", 4)) as ypool,
            tc.tile_pool(name="small", bufs=cfg.get("sbufs", 4)) as spool,
            tc.tile_pool(name="accp", bufs=1) as accpool,
        ):
            accs = accpool.tile([P, 2 * n_t], _DT)
            scrA = accpool.tile([P, M_MAX], _BF)
            scrB = accpool.tile([P, M_MAX], _BF)

            # dummy sqrt makes bacc's table pass load sqrt_and_others (which
            # also contains Square), avoiding a mid-stream ACT table load
            nc.vector.memset(scrA[:, :2], 1.0)
            nc.scalar.activation(
                out=scrA[:, :2],
                in_=scrA[:, :2],
                func=mybir.ActivationFunctionType.Sqrt,
            )

            def stage_a(i, col0, F):
                M = F // 3
                xt_full = xpool.tile([P, F_PAD], _DT, tag="xt")
                xt = xt_full[:, :F]
                nc.sync.dma_start(out=xt, in_=x[:, col0 : col0 + F])
                yt_full = ypool.tile([P, F_PAD], _BF, tag="yt")
                yt = yt_full[:, :F]
                # transposed write view: yv iterated (m, t) -> offset t*M + m
                yv = yt.rearrange("p (t m) -> p m t", t=3)
                se = sq_eng.get(i, "A")
                if se == "V":
                    nc.vector.tensor_tensor(
                        out=yv, in0=xt, in1=xt, op=mybir.AluOpType.mult
                    )
                elif se == "A":
                    nc.scalar.activation(
                        out=yv, in_=xt, func=mybir.ActivationFunctionType.Square
                    )
                else:  # float: ACT front fraction, DVE the rest
                    c = 3 * (int(F * se) // 12) * 4
                    nc.scalar.activation(
                        out=yv[:, : c // 3, :], in_=xt[:, :c],
                        func=mybir.ActivationFunctionType.Square,
                    )
                    nc.vector.tensor_tensor(
                        out=yv[:, c // 3 :, :], in0=xt[:, c:], in1=xt[:, c:],
                        op=mybir.AluOpType.mult,
                    )
                s_full = spool.tile([P, M_MAX], _BF, tag="s")
                s = s_full[:, :M]
                nc.vector.tensor_tensor(
                    out=s, in0=yt[:, :M], in1=yt[:, M : 2 * M],
                    op=mybir.AluOpType.add,
                )
                nc.vector.tensor_tensor(
                    out=s, in0=s, in1=yt[:, 2 * M : 3 * M],
                    op=mybir.AluOpType.add,
                )
                return s, M

            def stage_b(i, s, M):
                nc.vector.tensor_scalar(
                    out=scrA[:, :M],
                    in0=s,
                    scalar1=float(HALF_BODY * HALF_BODY),
                    scalar2=None,
                    op0=mybir.AluOpType.is_gt,
                    op1=mybir.AluOpType.add,
                    accum_out=accs[:, 2 * i + 1 : 2 * i + 2],
                )
                d_full = spool.tile([P, M_MAX], _BF, tag="d")
                d = d_full[:, :M]
                nc.scalar.activation(
                    out=d, in_=s, func=mybir.ActivationFunctionType.Sqrt
                )
                nc.vector.tensor_scalar(
                    out=scrB[:, :M],
                    in0=d,
                    scalar1=float(HALF_BODY),
                    scalar2=None,
                    op0=mybir.AluOpType.max,
                    op1=mybir.AluOpType.add,
                    accum_out=accs[:, 2 * i : 2 * i + 1],
                )

            from collections import deque

            pending = deque()
            col0 = 0
            for i, F in enumerate(tiles):
                s, M = stage_a(i, col0, F)
                col0 += F
                pending.append((i, s, M))
                eff_lag = 0 if (flush_from is not None and i >= flush_from) else b_lag
                while len(pending) > eff_lag:
                    stage_b(*pending.popleft())
            while pending:
                stage_b(*pending.popleft())

            nc.sync.dma_start(out=out[:, :], in_=accs)

    nc.compile()
    return nc


_nc_cache = None
last_results = None


def kernel(kps_world_pred: np.ndarray) -> np.ndarray:
    global _nc_cache, last_results
    x = np.ascontiguousarray(kps_world_pred, dtype=np.float32)
    assert x.shape == (B, J, D)

    shards = x.reshape(N_CORES, P, COLS)
    in_maps = [{"x": shards[c]} for c in range(N_CORES)]

    if _nc_cache is None:
        _nc_cache = build_nc()

    # the axon terminal occasionally reports a transient
    # NRT_EXEC_UNIT_UNRECOVERABLE left over from a previous run; it clears
    # after a short wait, so retry rather than fail the whole call
    import time

    res = None
    for attempt in range(3):
        try:
            res = run_bass_kernel_spmd(_nc_cache, in_maps, list(range(N_CORES)))
            break
        except Exception:
            if attempt == 2:
                raise
            time.sleep(15)
    last_results = res

    # per-partition device partials hold sum(max(d,1)) + count(s>1)
    #   = masked_sum + n_triplets, so subtract the global triplet count.
    total = np.float64(0.0)
    for c in range(N_CORES):
        total += res.results[c]["out"].astype(np.float64).sum()
    total -= np.float64(B * J)
    return np.asarray(total / (B * J), dtype=np.float32)


# revision 5
# speedup vs baseline: 1.0057x; 1.0057x over previous
"""Trainium2 Bass kernel for BodyStructureLoss.

Computes: mean over (B, J) of where(||kps[b,j,:]|| > 1.0, ||kps[b,j,:]||, 0)
for kps of shape [524288, 17, 3] float32.

Strategy (data-parallel over 8 NeuronCores):
  - Each core gets B/8 = 65536 batch rows viewed as [128, 26112] fp32.
  - Tiles of F columns stream in via DMA. Each tile is squared with a
    transposed write into a bf16 tile yt so the three components of each
    (x,y,z) triplet land in separate contiguous thirds:
        yt[:, t*M + m] = xt[:, 3m + t]^2   (M = F/3)
    The squaring engine alternates ACT (activation Square) / DVE
    (tensor_tensor mult) to balance load.
  - Two packed bf16 tensor_tensor adds (DVE 2x mode) produce the
    per-triplet squared norm s [P, M].
  - Per tile: tensor_scalar is_gt (4x bf16) accumulates count(s > 1) into
    accs[:, 2i+1]; ACT sqrt gives d bf16; tensor_scalar max (4x bf16)
    accumulates sum(max(d, 1)) into accs[:, 2i].
  - The tile plan ramps up (612, 1020) and tapers down (1224, 1020, 612,
    408) so the DMA stream stays packed and the post-stream compute drain
    is short.
  - Host sums all partials: sum(accs) = masked_sum + B*J, so subtract
    B*J and divide by B*J.
"""

import os

import numpy as np

# the NTFF trace path needs antenv.axon_hooks, which this client image lacks;
# force-disable so a stray BASS_TRACE=1 in the environment cannot break runs
os.environ["BASS_NEVER_TRACE"] = "1"

import concourse.bacc as bacc
import concourse.mybir as mybir
from concourse.bass_utils import run_bass_kernel_spmd
from concourse.tile import TileContext

B, J, D = 524288, 17, 3
HALF_BODY = 1.0  # threshold/2 with threshold=2.0
N_CORES = 8
P = 128
COLS = (B // N_CORES) * J * D // P  # 26112

_DT = mybir.dt.float32
_BF = mybir.dt.bfloat16


def _default_cfg():
    ramp = [612, 1020]
    tail = [1224, 1020, 816, 204]
    body = COLS - sum(ramp) - sum(tail)
    assert body % 1632 == 0
    tiles = ramp + [1632] * (body // 1632) + tail
    n = len(tiles)
    nt = len(tail)
    sq = {i: ("A" if i % 2 == 0 else "V") for i in range(n - nt)}
    for k, e in enumerate("AVAV"):
        sq[n - nt + k] = e
    return {"tiles": tiles, "sq_eng": sq, "b_lag": 2, "sbufs": 8}


def build_nc(cfg=None):
    if cfg is None:
        cfg = _default_cfg()
    tiles = cfg["tiles"]
    assert sum(tiles) == COLS
    assert all(f % 3 == 0 for f in tiles)
    n_t = len(tiles)
    M_MAX = max(tiles) // 3
    F_PAD = max(tiles)
    sq_eng = cfg.get("sq_eng", {})
    add_eng = cfg.get("add_eng", {})
    count_eng = cfg.get("count_eng", {})
    b_lag = cfg.get("b_lag", 1)
    flush_from = cfg.get("flush_from", None)

    nc = bacc.Bacc(
        "TRN2", target_bir_lowering=False, debug=False, num_devices=N_CORES
    )
    x = nc.dram_tensor("x", [P, COLS], _DT, kind="ExternalInput")
    out = nc.dram_tensor("out", [P, 2 * n_t], _DT, kind="ExternalOutput")

    with TileContext(nc, pool_alloc_mode=cfg.get("pool_mode", "stack")) as tc:
        with (
            tc.tile_pool(name="xin", bufs=cfg.get("xbufs", 6)) as xpool,
            tc.tile_pool(name="ysq", bufs=cfg.get("ybufs", 4)) as ypool,
            tc.tile_pool(name="small", bufs=cfg.get("sbufs", 4)) as spool,
            tc.tile_pool(name="accp", bufs=1) as accpool,
        ):
            accs = accpool.tile([P, 2 * n_t], _DT)
            scrA = accpool.tile([P, M_MAX], _BF)
            scrB = accpool.tile([P, M_MAX], _BF)

            # dummy sqrt makes bacc's table pass load sqrt_and_others (which
            # also contains Square), avoiding a mid-stream ACT table load
            nc.vector.memset(scrA[:, :2], 1.0)
            nc.scalar.activation(
                out=scrA[:, :2],
                in_=scrA[:, :2],
                func=mybir.ActivationFunctionType.Sqrt,
            )

            def stage_a(i, col0, F):
                M = F // 3
                xt_full = xpool.tile([P, F_PAD], _DT, tag="xt")
                xt = xt_full[:, :F]
                nc.sync.dma_start(out=xt, in_=x[:, col0 : col0 + F])
                yt_full = ypool.tile([P, F_PAD], _BF, tag="yt")
                yt = yt_full[:, :F]
                # transposed write view: yv iterated (m, t) -> offset t*M + m
                yv = yt.rearrange("p (t m) -> p m t", t=3)
                se = sq_eng.get(i, "A")
                if se == "V":
                    nc.vector.tensor_tensor(
                        out=yv, in0=xt, in1=xt, op=mybir.AluOpType.mult
                    )
                elif se == "A":
                    nc.scalar.activation(
                        out=yv, in_=xt, func=mybir.ActivationFunctionType.Square
                    )
                else:  # float: ACT front fraction, DVE the rest
                    c = 3 * (int(F * se) // 12) * 4
                    nc.scalar.activation(
                        out=yv[:, : c // 3, :], in_=xt[:, :c],
                        func=mybir.ActivationFunctionType.Square,
                    )
                    nc.vector.tensor_tensor(
                        out=yv[:, c // 3 :, :], in0=xt[:, c:], in1=xt[:, c:],
                        op=mybir.AluOpType.mult,
                    )
                s_full = spool.tile([P, M_MAX], _BF, tag="s")
                s = s_full[:, :M]
                nc.vector.tensor_tensor(
                    out=s, in0=yt[:, :M], in1=yt[:, M : 2 * M],
                    op=mybir.AluOpType.add,
                )
                nc.vector.tensor_tensor(
                    out=s, in0=s, in1=yt[:, 2 * M : 3 * M],
                    op=mybir.AluOpType.add,
                )
                return s, M

            def stage_b(i, s, M):
                nc.vector.tensor_scalar(
                    out=scrA[:, :M],
                    in0=s,
                    scalar1=float(HALF_BODY * HALF_BODY),
                    scalar2=None,
                    op0=mybir.AluOpType.is_gt,
                    op1=mybir.AluOpType.add,
                    accum_out=accs[:, 2 * i + 1 : 2 * i + 2],
                )
                d_full = spool.tile([P, M_MAX], _BF, tag="d")
                d = d_full[:, :M]
                nc.scalar.activation(
                    out=d, in_=s, func=mybir.ActivationFunctionType.Sqrt
                )
                nc.vector.tensor_scalar(
                    out=scrB[:, :M],
                    in0=d,
                    scalar1=float(HALF_BODY),
                    scalar2=None,
                    op0=mybir.AluOpType.max,
                    op1=mybir.AluOpType.add,
                    accum_out=accs[:, 2 * i : 2 * i + 1],
                )

            from collections import deque

            pending = deque()
            col0 = 0
            for i, F in enumerate(tiles):
                s, M = stage_a(i, col0, F)
                col0 += F
                pending.append((i, s, M))
                eff_lag = 0 if (flush_from is not None and i >= flush_from) else b_lag
                while len(pending) > eff_lag:
                    stage_b(*pending.popleft())
            while pending:
                stage_b(*pending.popleft())

            nc.sync.dma_start(out=out[:, :], in_=accs)

    nc.compile()
    return nc


_nc_cache = None
last_results = None


def kernel(kps_world_pred: np.ndarray) -> np.ndarray:
    global _nc_cache, last_results
    x = np.ascontiguousarray(kps_world_pred, dtype=np.float32)
    assert x.shape == (B, J, D)

    shards = x.reshape(N_CORES, P, COLS)
    in_maps = [{"x": shards[c]} for c in range(N_CORES)]

    if _nc_cache is None:
        _nc_cache = build_nc()

    # the axon terminal occasionally reports a transient
    # NRT_EXEC_UNIT_UNRECOVERABLE left over from a previous run; it clears
    # after a short wait, so retry rather than fail the whole call
    import time

    res = None
    for attempt in range(3):
        try:
            res = run_bass_kernel_spmd(_nc_cache, in_maps, list(range(N_CORES)))
            break
        except Exception:
            if attempt == 2:
                raise
            time.sleep(15)
    last_results = res

    # per-partition device partials hold sum(max(d,1)) + count(s>1)
    #   = masked_sum + n_triplets, so subtract the global triplet count.
    total = np.float64(0.0)
    for c in range(N_CORES):
        total += res.results[c]["out"].astype(np.float64).sum()
    total -= np.float64(B * J)
    return np.asarray(total / (B * J), dtype=np.float32)


# revision 7
# speedup vs baseline: 1.0099x; 1.0042x over previous
"""Trainium2 Bass kernel for BodyStructureLoss.

Computes: mean over (B, J) of where(||kps[b,j,:]|| > 1.0, ||kps[b,j,:]||, 0)
for kps of shape [524288, 17, 3] float32.

Strategy (data-parallel over 8 NeuronCores):
  - Each core gets B/8 = 65536 batch rows viewed as [128, 26112] fp32.
  - Tiles of F columns stream in via DMA. Each tile is squared with a
    transposed write into a bf16 tile yt so the three components of each
    (x,y,z) triplet land in separate contiguous thirds:
        yt[:, t*M + m] = xt[:, 3m + t]^2   (M = F/3)
    The squaring engine alternates ACT (activation Square) / DVE
    (tensor_tensor mult) to balance load.
  - Two packed bf16 tensor_tensor adds (DVE 2x mode) produce the
    per-triplet squared norm s [P, M].
  - Per tile: tensor_scalar is_gt (4x bf16) accumulates count(s > 1) into
    accs[:, 2i+1]; ACT sqrt gives d bf16; tensor_scalar max (4x bf16)
    accumulates sum(max(d, 1)) into accs[:, 2i].
  - The tile plan ramps up (612, 1020) and tapers down (1122, 918, 816,
    408) so the DMA stream stays packed and the post-stream compute drain
    is short.
  - Host sums all partials: sum(accs) = masked_sum + B*J, so subtract
    B*J and divide by B*J.
"""

import os

import numpy as np

# the NTFF trace path needs antenv.axon_hooks, which this client image lacks;
# force-disable so a stray BASS_TRACE=1 in the environment cannot break runs
os.environ["BASS_NEVER_TRACE"] = "1"

import concourse.bacc as bacc
import concourse.mybir as mybir
from concourse.bass_utils import run_bass_kernel_spmd
from concourse.tile import TileContext

B, J, D = 524288, 17, 3
HALF_BODY = 1.0  # threshold/2 with threshold=2.0
N_CORES = 8
P = 128
COLS = (B // N_CORES) * J * D // P  # 26112

_DT = mybir.dt.float32
_BF = mybir.dt.bfloat16


def _default_cfg():
    ramp = [612, 1020]
    tail = [1122, 918, 816, 408]
    body = COLS - sum(ramp) - sum(tail)
    assert body % 1632 == 0
    tiles = ramp + [1632] * (body // 1632) + tail
    n = len(tiles)
    nt = len(tail)
    sq = {i: ("A" if i % 2 == 0 else "V") for i in range(n - nt)}
    for k, e in enumerate("AVAV"):
        sq[n - nt + k] = e
    return {"tiles": tiles, "sq_eng": sq, "b_lag": 2, "sbufs": 10}


def build_nc(cfg=None):
    if cfg is None:
        cfg = _default_cfg()
    tiles = cfg["tiles"]
    assert sum(tiles) == COLS
    assert all(f % 3 == 0 for f in tiles)
    n_t = len(tiles)
    M_MAX = max(tiles) // 3
    F_PAD = max(tiles)
    sq_eng = cfg.get("sq_eng", {})
    add_eng = cfg.get("add_eng", {})
    count_eng = cfg.get("count_eng", {})
    b_lag = cfg.get("b_lag", 1)
    flush_from = cfg.get("flush_from", None)

    nc = bacc.Bacc(
        "TRN2", target_bir_lowering=False, debug=False, num_devices=N_CORES
    )
    x = nc.dram_tensor("x", [P, COLS], _DT, kind="ExternalInput")
    out = nc.dram_tensor("out", [P, 2 * n_t], _DT, kind="ExternalOutput")

    with TileContext(nc, pool_alloc_mode=cfg.get("pool_mode", "stack")) as tc:
        with (
            tc.tile_pool(name="xin", bufs=cfg.get("xbufs", 6)) as xpool,
            tc.tile_pool(name="ysq", bufs=cfg.get("ybufs", 4)) as ypool,
            tc.tile_pool(name="small", bufs=cfg.get("sbufs", 4)) as spool,
            tc.tile_pool(name="accp", bufs=1) as accpool,
        ):
            accs = accpool.tile([P, 2 * n_t], _DT)
            scrA = accpool.tile([P, M_MAX], _BF)
            scrB = accpool.tile([P, M_MAX], _BF)

            # dummy sqrt makes bacc's table pass load sqrt_and_others (which
            # also contains Square), avoiding a mid-stream ACT table load
            nc.vector.memset(scrA[:, :2], 1.0)
            nc.scalar.activation(
                out=scrA[:, :2],
                in_=scrA[:, :2],
                func=mybir.ActivationFunctionType.Sqrt,
            )

            def stage_a(i, col0, F):
                M = F // 3
                xt_full = xpool.tile([P, F_PAD], _DT, tag="xt")
                xt = xt_full[:, :F]
                nc.sync.dma_start(out=xt, in_=x[:, col0 : col0 + F])
                yt_full = ypool.tile([P, F_PAD], _BF, tag="yt")
                yt = yt_full[:, :F]
                # transposed write view: yv iterated (m, t) -> offset t*M + m
                yv = yt.rearrange("p (t m) -> p m t", t=3)
                se = sq_eng.get(i, "A")
                if se == "V":
                    nc.vector.tensor_tensor(
                        out=yv, in0=xt, in1=xt, op=mybir.AluOpType.mult
                    )
                elif se == "A":
                    nc.scalar.activation(
                        out=yv, in_=xt, func=mybir.ActivationFunctionType.Square
                    )
                else:  # float: ACT front fraction, DVE the rest
                    c = 3 * (int(F * se) // 12) * 4
                    nc.scalar.activation(
                        out=yv[:, : c // 3, :], in_=xt[:, :c],
                        func=mybir.ActivationFunctionType.Square,
                    )
                    nc.vector.tensor_tensor(
                        out=yv[:, c // 3 :, :], in0=xt[:, c:], in1=xt[:, c:],
                        op=mybir.AluOpType.mult,
                    )
                s_full = spool.tile([P, M_MAX], _BF, tag="s")
                s = s_full[:, :M]
                nc.vector.tensor_tensor(
                    out=s, in0=yt[:, :M], in1=yt[:, M : 2 * M],
                    op=mybir.AluOpType.add,
                )
                nc.vector.tensor_tensor(
                    out=s, in0=s, in1=yt[:, 2 * M : 3 * M],
                    op=mybir.AluOpType.add,
                )
                return s, M

            def stage_b(i, s, M):
                nc.vector.tensor_scalar(
                    out=scrA[:, :M],
                    in0=s,
                    scalar1=float(HALF_BODY * HALF_BODY),
                    scalar2=None,
                    op0=mybir.AluOpType.is_gt,
                    op1=mybir.AluOpType.add,
                    accum_out=accs[:, 2 * i + 1 : 2 * i + 2],
                )
                d_full = spool.tile([P, M_MAX], _BF, tag="d")
                d = d_full[:, :M]
                nc.scalar.activation(
                    out=d, in_=s, func=mybir.ActivationFunctionType.Sqrt
                )
                nc.vector.tensor_scalar(
                    out=scrB[:, :M],
                    in0=d,
                    scalar1=float(HALF_BODY),
                    scalar2=None,
                    op0=mybir.AluOpType.max,
                    op1=mybir.AluOpType.add,
                    accum_out=accs[:, 2 * i : 2 * i + 1],
                )

            from collections import deque

            pending = deque()
            col0 = 0
            for i, F in enumerate(tiles):
                s, M = stage_a(i, col0, F)
                col0 += F
                pending.append((i, s, M))
                eff_lag = 0 if (flush_from is not None and i >= flush_from) else b_lag
                while len(pending) > eff_lag:
                    stage_b(*pending.popleft())
            while pending:
                stage_b(*pending.popleft())

            nc.sync.dma_start(out=out[:, :], in_=accs)

    nc.compile()
    return nc


_nc_cache = None
last_results = None


def kernel(kps_world_pred: np.ndarray) -> np.ndarray:
    global _nc_cache, last_results
    x = np.ascontiguousarray(kps_world_pred, dtype=np.float32)
    assert x.shape == (B, J, D)

    shards = x.reshape(N_CORES, P, COLS)
    in_maps = [{"x": shards[c]} for c in range(N_CORES)]

    if _nc_cache is None:
        _nc_cache = build_nc()

    # the axon terminal occasionally reports a transient
    # NRT_EXEC_UNIT_UNRECOVERABLE left over from a previous run; it clears
    # after a short wait, so retry rather than fail the whole call
    import time

    res = None
    for attempt in range(3):
        try:
            res = run_bass_kernel_spmd(_nc_cache, in_maps, list(range(N_CORES)))
            break
        except Exception:
            if attempt == 2:
                raise
            time.sleep(15)
    last_results = res

    # per-partition device partials hold sum(max(d,1)) + count(s>1)
    #   = masked_sum + n_triplets, so subtract the global triplet count.
    total = np.float64(0.0)
    for c in range(N_CORES):
        total += res.results[c]["out"].astype(np.float64).sum()
    total -= np.float64(B * J)
    return np.asarray(total / (B * J), dtype=np.float32)


# revision 9
# speedup vs baseline: 1.0139x; 1.0039x over previous
"""Trainium2 Bass kernel for BodyStructureLoss.

Computes: mean over (B, J) of where(||kps[b,j,:]|| > 1.0, ||kps[b,j,:]||, 0)
for kps of shape [524288, 17, 3] float32.

Strategy (data-parallel over 8 NeuronCores):
  - Each core gets B/8 = 65536 batch rows viewed as [128, 26112] fp32.
  - Tiles of F columns stream in via DMA. Each tile is squared with a
    transposed write into a bf16 tile yt so the three components of each
    (x,y,z) triplet land in separate contiguous thirds:
        yt[:, t*M + m] = xt[:, 3m + t]^2   (M = F/3)
    The squaring engine alternates ACT (activation Square) / DVE
    (tensor_tensor mult) to balance load.
  - Two packed bf16 tensor_tensor adds (DVE 2x mode) produce the
    per-triplet squared norm s [P, M].
  - Per tile: tensor_scalar is_gt (4x bf16) accumulates count(s > 1) into
    accs[:, 2i+1]; ACT sqrt gives d bf16; tensor_scalar max (4x bf16)
    accumulates sum(max(d, 1)) into accs[:, 2i].
  - The tile plan ramps up (612, 1020) and tapers down (1326, 918, 612,
    408) so the DMA stream stays packed and the post-stream compute drain
    is short.
  - Host sums all partials: sum(accs) = masked_sum + B*J, so subtract
    B*J and divide by B*J.
"""

import os

import numpy as np

# the NTFF trace path needs antenv.axon_hooks, which this client image lacks;
# force-disable so a stray BASS_TRACE=1 in the environment cannot break runs
os.environ["BASS_NEVER_TRACE"] = "1"

import concourse.bacc as bacc
import concourse.mybir as mybir
from concourse.bass_utils import run_bass_kernel_spmd
from concourse.tile import TileContext

B, J, D = 524288, 17, 3
HALF_BODY = 1.0  # threshold/2 with threshold=2.0
N_CORES = 8
P = 128
COLS = (B // N_CORES) * J * D // P  # 26112

_DT = mybir.dt.float32
_BF = mybir.dt.bfloat16


def _default_cfg():
    ramp = [612, 1020]
    tail = [1326, 918, 612, 408]
    body = COLS - sum(ramp) - sum(tail)
    assert body % 1632 == 0
    tiles = ramp + [1632] * (body // 1632) + tail
    n = len(tiles)
    nt = len(tail)
    sq = {i: ("A" if i % 2 == 0 else "V") for i in range(n - nt)}
    for k, e in enumerate("AVAA"):
        sq[n - nt + k] = e
    return {"tiles": tiles, "sq_eng": sq, "b_lag": 2, "sbufs": 10}


def build_nc(cfg=None):
    if cfg is None:
        cfg = _default_cfg()
    tiles = cfg["tiles"]
    assert sum(tiles) == COLS
    assert all(f % 3 == 0 for f in tiles)
    n_t = len(tiles)
    M_MAX = max(tiles) // 3
    F_PAD = max(tiles)
    sq_eng = cfg.get("sq_eng", {})
    add_eng = cfg.get("add_eng", {})
    count_eng = cfg.get("count_eng", {})
    b_lag = cfg.get("b_lag", 1)
    flush_from = cfg.get("flush_from", None)

    nc = bacc.Bacc(
        "TRN2", target_bir_lowering=False, debug=False, num_devices=N_CORES
    )
    x = nc.dram_tensor("x", [P, COLS], _DT, kind="ExternalInput")
    out = nc.dram_tensor("out", [P, 2 * n_t], _DT, kind="ExternalOutput")

    with TileContext(nc, pool_alloc_mode=cfg.get("pool_mode", "stack")) as tc:
        with (
            tc.tile_pool(name="xin", bufs=cfg.get("xbufs", 6)) as xpool,
            tc.tile_pool(name="ysq", bufs=cfg.get("ybufs", 4)) as ypool,
            tc.tile_pool(name="small", bufs=cfg.get("sbufs", 4)) as spool,
            tc.tile_pool(name="accp", bufs=1) as accpool,
        ):
            accs = accpool.tile([P, 2 * n_t], _DT)
            scrA = accpool.tile([P, M_MAX], _BF)
            scrB = accpool.tile([P, M_MAX], _BF)

            # dummy sqrt makes bacc's table pass load sqrt_and_others (which
            # also contains Square), avoiding a mid-stream ACT table load
            nc.vector.memset(scrA[:, :2], 1.0)
            nc.scalar.activation(
                out=scrA[:, :2],
                in_=scrA[:, :2],
                func=mybir.ActivationFunctionType.Sqrt,
            )

            def stage_a(i, col0, F):
                M = F // 3
                xt_full = xpool.tile([P, F_PAD], _DT, tag="xt")
                xt = xt_full[:, :F]
                nc.sync.dma_start(out=xt, in_=x[:, col0 : col0 + F])
                yt_full = ypool.tile([P, F_PAD], _BF, tag="yt")
                yt = yt_full[:, :F]
                # transposed write view: yv iterated (m, t) -> offset t*M + m
                yv = yt.rearrange("p (t m) -> p m t", t=3)
                se = sq_eng.get(i, "A")
                if se == "V":
                    nc.vector.tensor_tensor(
                        out=yv, in0=xt, in1=xt, op=mybir.AluOpType.mult
                    )
                elif se == "A":
                    nc.scalar.activation(
                        out=yv, in_=xt, func=mybir.ActivationFunctionType.Square
                    )
                else:  # float: ACT front fraction, DVE the rest
                    c = 3 * (int(F * se) // 12) * 4
                    nc.scalar.activation(
                        out=yv[:, : c // 3, :], in_=xt[:, :c],
                        func=mybir.ActivationFunctionType.Square,
                    )
                    nc.vector.tensor_tensor(
                        out=yv[:, c // 3 :, :], in0=xt[:, c:], in1=xt[:, c:],
                        op=mybir.AluOpType.mult,
                    )
                s_full = spool.tile([P, M_MAX], _BF, tag="s")
                s = s_full[:, :M]
                nc.vector.tensor_tensor(
                    out=s, in0=yt[:, :M], in1=yt[:, M : 2 * M],
                    op=mybir.AluOpType.add,
                )
                nc.vector.tensor_tensor(
                    out=s, in0=s, in1=yt[:, 2 * M : 3 * M],
                    op=mybir.AluOpType.add,
                )
                return s, M

            def stage_b(i, s, M):
                nc.vector.tensor_scalar(
                    out=scrA[:, :M],
                    in0=s,
                    scalar1=float(HALF_BODY * HALF_BODY),
                    scalar2=None,
                    op0=mybir.AluOpType.is_gt,
                    op1=mybir.AluOpType.add,
                    accum_out=accs[:, 2 * i + 1 : 2 * i + 2],
                )
                d_full = spool.tile([P, M_MAX], _BF, tag="d")
                d = d_full[:, :M]
                nc.scalar.activation(
                    out=d, in_=s, func=mybir.ActivationFunctionType.Sqrt
                )
                nc.vector.tensor_scalar(
                    out=scrB[:, :M],
                    in0=d,
                    scalar1=float(HALF_BODY),
                    scalar2=None,
                    op0=mybir.AluOpType.max,
                    op1=mybir.AluOpType.add,
                    accum_out=accs[:, 2 * i : 2 * i + 1],
                )

            from collections import deque

            pending = deque()
            col0 = 0
            for i, F in enumerate(tiles):
                s, M = stage_a(i, col0, F)
                col0 += F
                pending.append((i, s, M))
                eff_lag = 0 if (flush_from is not None and i >= flush_from) else b_lag
                while len(pending) > eff_lag:
                    stage_b(*pending.popleft())
            while pending:
                stage_b(*pending.popleft())

            nc.sync.dma_start(out=out[:, :], in_=accs)

    nc.compile()
    return nc


_nc_cache = None
last_results = None


def kernel(kps_world_pred: np.ndarray) -> np.ndarray:
    global _nc_cache, last_results
    x = np.ascontiguousarray(kps_world_pred, dtype=np.float32)
    assert x.shape == (B, J, D)

    shards = x.reshape(N_CORES, P, COLS)
    in_maps = [{"x": shards[c]} for c in range(N_CORES)]

    if _nc_cache is None:
        _nc_cache = build_nc()

    # the axon terminal occasionally reports a transient
    # NRT_EXEC_UNIT_UNRECOVERABLE left over from a previous run; it clears
    # after a short wait, so retry rather than fail the whole call
    import time

    res = None
    for attempt in range(3):
        try:
            res = run_bass_kernel_spmd(_nc_cache, in_maps, list(range(N_CORES)))
            break
        except Exception:
            if attempt == 2:
                raise
            time.sleep(15)
    last_results = res

    # per-partition device partials hold sum(max(d,1)) + count(s>1)
    #   = masked_sum + n_triplets, so subtract the global triplet count.
    total = np.float64(0.0)
    for c in range(N_CORES):
        total += res.results[c]["out"].astype(np.float64).sum()
    total -= np.float64(B * J)
    return np.asarray(total / (B * J), dtype=np.float32)


# revision 11
# speedup vs baseline: 1.0150x; 1.0011x over previous
"""Trainium2 Bass kernel for BodyStructureLoss.

Computes: mean over (B, J) of where(||kps[b,j,:]|| > 1.0, ||kps[b,j,:]||, 0)
for kps of shape [524288, 17, 3] float32.

Strategy (data-parallel over 8 NeuronCores):
  - Each core gets B/8 = 65536 batch rows viewed as [128, 26112] fp32.
  - Tiles of F columns stream in via DMA. Each tile is squared with a
    transposed write into a bf16 tile yt so the three components of each
    (x,y,z) triplet land in separate contiguous thirds:
        yt[:, t*M + m] = xt[:, 3m + t]^2   (M = F/3)
    The squaring engine alternates ACT (activation Square) / DVE
    (tensor_tensor mult) to balance load.
  - Two packed bf16 tensor_tensor adds (DVE 2x mode) produce the
    per-triplet squared norm s [P, M].
  - Per tile: tensor_scalar is_gt (4x bf16) accumulates count(s > 1) into
    accs[:, 2i+1]; ACT sqrt gives d bf16; tensor_scalar max (4x bf16)
    accumulates sum(max(d, 1)) into accs[:, 2i].
  - The tile plan ramps up (612, 1020) and tapers down (1326, 918, 612,
    408) so the DMA stream stays packed and the post-stream compute drain
    is short.
  - Host sums all partials: sum(accs) = masked_sum + B*J, so subtract
    B*J and divide by B*J.
"""

import os

import numpy as np

# the NTFF trace path needs antenv.axon_hooks, which this client image lacks;
# force-disable so a stray BASS_TRACE=1 in the environment cannot break runs
os.environ["BASS_NEVER_TRACE"] = "1"

import concourse.bacc as bacc
import concourse.mybir as mybir
from concourse.bass_utils import run_bass_kernel_spmd
from concourse.tile import TileContext

B, J, D = 524288, 17, 3
HALF_BODY = 1.0  # threshold/2 with threshold=2.0
N_CORES = 8
P = 128
COLS = (B // N_CORES) * J * D // P  # 26112

_DT = mybir.dt.float32
_BF = mybir.dt.bfloat16


def _default_cfg():
    ramp = [612, 1020]
    tail = [1326, 918, 612, 408]
    body = COLS - sum(ramp) - sum(tail)
    assert body % 1632 == 0
    tiles = ramp + [1632] * (body // 1632) + tail
    n = len(tiles)
    nt = len(tail)
    sq = {i: ("A" if i % 2 == 0 else "V") for i in range(n - nt)}
    for k, e in enumerate("AVAA"):
        sq[n - nt + k] = e
    return {"tiles": tiles, "sq_eng": sq, "b_lag": 2, "sbufs": 10}


def build_nc(cfg=None):
    if cfg is None:
        cfg = _default_cfg()
    tiles = cfg["tiles"]
    assert sum(tiles) == COLS
    assert all(f % 3 == 0 for f in tiles)
    n_t = len(tiles)
    M_MAX = max(tiles) // 3
    F_PAD = max(tiles)
    sq_eng = cfg.get("sq_eng", {})
    add_eng = cfg.get("add_eng", {})
    count_eng = cfg.get("count_eng", {})
    b_lag = cfg.get("b_lag", 1)
    flush_from = cfg.get("flush_from", None)

    nc = bacc.Bacc(
        "TRN2", target_bir_lowering=False, debug=False, num_devices=N_CORES
    )
    x = nc.dram_tensor("x", [P, COLS], _DT, kind="ExternalInput")
    # bf16 accumulator columns: ~1 ulp rounding per ~500-magnitude column
    # partial (relative error ~5e-5 on the final mean), halves the output
    # DMA transfer
    out = nc.dram_tensor("out", [P, 2 * n_t], _BF, kind="ExternalOutput")

    with TileContext(nc, pool_alloc_mode=cfg.get("pool_mode", "stack")) as tc:
        with (
            tc.tile_pool(name="xin", bufs=cfg.get("xbufs", 6)) as xpool,
            tc.tile_pool(name="ysq", bufs=cfg.get("ybufs", 4)) as ypool,
            tc.tile_pool(name="small", bufs=cfg.get("sbufs", 4)) as spool,
            tc.tile_pool(name="accp", bufs=1) as accpool,
        ):
            accs = accpool.tile([P, 2 * n_t], _BF)
            scrA = accpool.tile([P, M_MAX], _BF)
            scrB = accpool.tile([P, M_MAX], _BF)

            # dummy sqrt makes bacc's table pass load sqrt_and_others (which
            # also contains Square), avoiding a mid-stream ACT table load
            nc.vector.memset(scrA[:, :2], 1.0)
            nc.scalar.activation(
                out=scrA[:, :2],
                in_=scrA[:, :2],
                func=mybir.ActivationFunctionType.Sqrt,
            )

            def stage_a(i, col0, F):
                M = F // 3
                xt_full = xpool.tile([P, F_PAD], _DT, tag="xt")
                xt = xt_full[:, :F]
                nc.sync.dma_start(out=xt, in_=x[:, col0 : col0 + F])
                yt_full = ypool.tile([P, F_PAD], _BF, tag="yt")
                yt = yt_full[:, :F]
                # transposed write view: yv iterated (m, t) -> offset t*M + m
                yv = yt.rearrange("p (t m) -> p m t", t=3)
                se = sq_eng.get(i, "A")
                if se == "V":
                    nc.vector.tensor_tensor(
                        out=yv, in0=xt, in1=xt, op=mybir.AluOpType.mult
                    )
                elif se == "A":
                    nc.scalar.activation(
                        out=yv, in_=xt, func=mybir.ActivationFunctionType.Square
                    )
                else:  # float: ACT front fraction, DVE the rest
                    c = 3 * (int(F * se) // 12) * 4
                    nc.scalar.activation(
                        out=yv[:, : c // 3, :], in_=xt[:, :c],
                        func=mybir.ActivationFunctionType.Square,
                    )
                    nc.vector.tensor_tensor(
                        out=yv[:, c // 3 :, :], in0=xt[:, c:], in1=xt[:, c:],
                        op=mybir.AluOpType.mult,
                    )
                s_full = spool.tile([P, M_MAX], _BF, tag="s")
                s = s_full[:, :M]
                nc.vector.tensor_tensor(
                    out=s, in0=yt[:, :M], in1=yt[:, M : 2 * M],
                    op=mybir.AluOpType.add,
                )
                nc.vector.tensor_tensor(
                    out=s, in0=s, in1=yt[:, 2 * M : 3 * M],
                    op=mybir.AluOpType.add,
                )
                return s, M

            def stage_b(i, s, M):
                nc.vector.tensor_scalar(
                    out=scrA[:, :M],
                    in0=s,
                    scalar1=float(HALF_BODY * HALF_BODY),
                    scalar2=None,
                    op0=mybir.AluOpType.is_gt,
                    op1=mybir.AluOpType.add,
                    accum_out=accs[:, 2 * i + 1 : 2 * i + 2],
                )
                d_full = spool.tile([P, M_MAX], _BF, tag="d")
                d = d_full[:, :M]
                nc.scalar.activation(
                    out=d, in_=s, func=mybir.ActivationFunctionType.Sqrt
                )
                nc.vector.tensor_scalar(
                    out=scrB[:, :M],
                    in0=d,
                    scalar1=float(HALF_BODY),
                    scalar2=None,
                    op0=mybir.AluOpType.max,
                    op1=mybir.AluOpType.add,
                    accum_out=accs[:, 2 * i : 2 * i + 1],
                )

            from collections import deque

            with nc.allow_low_precision("bf16 accs partials, ~5e-5 rel err"):
                pending = deque()
                col0 = 0
                for i, F in enumerate(tiles):
                    s, M = stage_a(i, col0, F)
                    col0 += F
                    pending.append((i, s, M))
                    eff_lag = 0 if (flush_from is not None and i >= flush_from) else b_lag
                    while len(pending) > eff_lag:
                        stage_b(*pending.popleft())
                while pending:
                    stage_b(*pending.popleft())

            nc.sync.dma_start(out=out[:, :], in_=accs)

    nc.compile()
    return nc


_nc_cache = None
last_results = None


def kernel(kps_world_pred: np.ndarray) -> np.ndarray:
    global _nc_cache, last_results
    x = np.ascontiguousarray(kps_world_pred, dtype=np.float32)
    assert x.shape == (B, J, D)

    shards = x.reshape(N_CORES, P, COLS)
    in_maps = [{"x": shards[c]} for c in range(N_CORES)]

    if _nc_cache is None:
        _nc_cache = build_nc()

    # the axon terminal occasionally fails transiently: either a raised
    # NRT_EXEC_UNIT_UNRECOVERABLE left over from a previous run, or a wedged
    # device silently returning non-finite garbage. Both clear after a short
    # wait, so retry rather than fail the whole call.
    import time

    res = None
    total = np.float64(0.0)
    for attempt in range(4):
        try:
            res = run_bass_kernel_spmd(_nc_cache, in_maps, list(range(N_CORES)))
        except Exception:
            if attempt == 3:
                raise
            time.sleep(15)
            continue
        partials = np.stack(
            [res.results[c]["out"].astype(np.float64) for c in range(N_CORES)]
        )
        if np.isfinite(partials).all():
            total = partials.sum()
            break
        if attempt == 3:
            total = partials.sum()  # give up; return what we got
            break
        time.sleep(15)
    last_results = res

    # per-partition device partials hold sum(max(d,1)) + count(s>1)
    #   = masked_sum + n_triplets, so subtract the global triplet count.
    total -= np.float64(B * J)
    return np.asarray(total / (B * J), dtype=np.float32)
